# revision 1
# baseline (speedup 1.0000x reference)
"""Trainium2 Bass kernel for nn_Encoder_88656714924838 (6-layer dense
transformer encoder with distance-bias attention, d_model=64, 4 heads).

Sharding: pure data parallel — batch 256 split as 32 per core across 8 cores.
Weights replicated. Host precomputes the (input-dependent) embedding gathers
and the exp(distance-bias + pad-mask) tensor; the device kernel runs all six
encoder layers (QKV projections, attention, softmax, FFN, layernorms).
"""

import sys

for _p in ("/opt/trn_rl_repo",):
    if _p not in sys.path:
        sys.path.insert(0, _p)

import numpy as np

D_MODEL = 64
N_HEADS = 4
D_K = 16
D_FF = 512
N_LAYERS = 6
B, L = 256, 128
N_CORES = 8
B_LOC = B // N_CORES
SCALE = 1.0 / np.sqrt(np.float32(D_K))


def _positional_encoding(length=L, d_model=D_MODEL):
    pos = np.arange(length, dtype=np.float32)[:, None]
    div = np.exp(
        np.arange(0, d_model, 2, dtype=np.float32) * (-np.log(10000.0) / d_model)
    )
    pe = np.zeros((length, d_model), dtype=np.float32)
    pe[:, 0::2] = np.sin(pos * div)
    pe[:, 1::2] = np.cos(pos * div)
    return pe


def _layernorm_emit(nc, pool_small, eps_tile, out_ap, v_ap, mybir):
    """Emit LN over free dim (64) of v_ap [128, 64] -> out_ap."""
    f32 = mybir.dt.float32
    stats = pool_small.tile([128, 6], f32, tag="ln_stats")
    nc.vector.bn_stats(out=stats[:], in_=v_ap)
    mv = pool_small.tile([128, 2], f32, tag="ln_mv")
    nc.vector.bn_aggr(out=mv[:], in_=stats[:])
    std = pool_small.tile([128, 1], f32, tag="ln_std")
    nc.scalar.activation(
        out=std[:],
        in_=mv[:, 1:2],
        func=mybir.ActivationFunctionType.Sqrt,
        bias=eps_tile[:, 0:1],
        scale=1.0,
    )
    rstd = pool_small.tile([128, 1], f32, tag="ln_rstd")
    nc.vector.reciprocal(out=rstd[:], in_=std[:])
    nmr = pool_small.tile([128, 1], f32, tag="ln_nmr")
    nc.vector.tensor_scalar(
        out=nmr[:], in0=mv[:, 0:1], scalar1=rstd[:, 0:1], scalar2=-1.0,
        op0=mybir.AluOpType.mult, op1=mybir.AluOpType.mult,
    )
    nc.scalar.activation(
        out=out_ap,
        in_=v_ap,
        func=mybir.ActivationFunctionType.Identity,
        bias=nmr[:, 0:1],
        scale=rstd[:, 0:1],
    )




def _layernorm_group_emit(nc, pool, eps_tile, out_aps, v_g, G, mybir, on_dve=False):
    """LN over free dim 64 for G batches at once: per-b bn_stats/aggr, then
    one batched sqrt/recip/scale chain, then per-b apply."""
    f32 = mybir.dt.float32
    stats_g = pool.tile([128, G, 6], f32, tag="ln_stats_g", bufs=2)
    mv_g = pool.tile([128, G, 2], f32, tag="ln_mv_g", bufs=2)
    for j in range(G):
        nc.vector.bn_stats(out=stats_g[:, j, :], in_=v_g[:, j, :])
        nc.vector.bn_aggr(out=mv_g[:, j, :], in_=stats_g[:, j, :])
    std_g = pool.tile([128, G, 1], f32, tag="ln_std_g", bufs=2)
    nc.scalar.activation(
        out=std_g[:], in_=mv_g[:, :, 1:2],
        func=mybir.ActivationFunctionType.Sqrt, bias=eps_tile[:, 0:1], scale=1.0,
    )
    rstd_g = pool.tile([128, G, 1], f32, tag="ln_rstd_g", bufs=2)
    nc.vector.reciprocal(out=rstd_g[:], in_=std_g[:])
    nmr_g = pool.tile([128, G, 1], f32, tag="ln_nmr_g", bufs=2)
    nc.vector.tensor_mul(out=nmr_g[:], in0=mv_g[:, :, 0:1], in1=rstd_g[:])
    nc.vector.tensor_scalar_mul(nmr_g[:], nmr_g[:], -1.0)
    for j in range(G):
        if on_dve:
            nc.vector.tensor_scalar(
                out=out_aps[j], in0=v_g[:, j, :],
                scalar1=rstd_g[:, j, 0:1], scalar2=nmr_g[:, j, 0:1],
                op0=mybir.AluOpType.mult, op1=mybir.AluOpType.add,
            )
        else:
            nc.scalar.activation(
                out=out_aps[j], in_=v_g[:, j, :],
                func=mybir.ActivationFunctionType.Identity,
                bias=nmr_g[:, j, 0:1], scale=rstd_g[:, j, 0:1],
            )

def _split_multi_waits(nc):
    """The walrus build here accepts only ONE sync-wait per instruction.
    Hoist extra semaphore waits onto same-engine NoOps placed just before
    the carrying instruction (equivalent: all waits still gate it)."""
    import concourse.mybir as mybir

    k = 0
    for fn in nc.m.functions:
        for blk in fn.blocks:
            new = []
            changed = False
            for inst in blk.instructions:
                si = inst.sync_info
                waits = list(si.on_wait) if (si and si.on_wait) else []
                if len(waits) > 1:
                    changed = True
                    for w in waits[:-1]:
                        k += 1
                        nop = mybir.InstNoOp(name=f"ws-{k}", ins=[], outs=[])
                        nop.engine = inst.engine
                        nop.sync_info = mybir.SyncInfo(on_wait=[w], on_update=[])
                        nc.register_instruction(nop)
                        new.append(nop)
                    si.on_wait = waits[-1:]
                new.append(inst)
            if changed:
                blk.instructions = new


def build_nc(n_layers=N_LAYERS, b_loc=B_LOC):
    """Build the Bass module. Same program runs on every core (SPMD)."""
    import concourse.bass as bass
    import concourse.mybir as mybir
    import concourse.tile as tile
    from concourse.masks import make_identity

    f32 = mybir.dt.float32
    bf16 = mybir.dt.bfloat16

    nc = bass.Bass("TRN2", target_bir_lowering=False, debug=False)

    x0_d = nc.dram_tensor("x0", [b_loc, L, D_MODEL], f32, kind="ExternalInput")
    ebt_d = nc.dram_tensor("ebt", [b_loc, L, N_HEADS, L], bf16, kind="ExternalInput")
    # wq is expanded on host to [.., 4, 64]: block h keeps only head h's 16
    # output cols (rest zero). Q^T_h then has zeros outside head h's rows, so
    # the scores matmul can contract over the full K=64 at base partition 0
    # (mixed PE row-base tile_positions crash this runtime).
    wq_d = nc.dram_tensor("wq", [n_layers, D_MODEL, 4, 64], bf16, kind="ExternalInput")
    wk_d = nc.dram_tensor("wk", [n_layers, D_MODEL, D_MODEL], bf16, kind="ExternalInput")
    wv_d = nc.dram_tensor("wv", [n_layers, D_MODEL, D_MODEL], bf16, kind="ExternalInput")
    wo_d = nc.dram_tensor("wo", [n_layers, D_MODEL, D_MODEL], f32, kind="ExternalInput")
    w1_d = nc.dram_tensor("w1", [n_layers, D_MODEL, D_FF], bf16, kind="ExternalInput")
    w2_d = nc.dram_tensor(
        "w2", [n_layers, 4, 128, D_MODEL], bf16, kind="ExternalInput"
    )
    out_d = nc.dram_tensor("out", [b_loc, L, D_MODEL], f32, kind="ExternalOutput")

    with tile.TileContext(nc) as tc:
        with (
            tc.tile_pool(name="consts", bufs=1) as consts,
            tc.tile_pool(name="state", bufs=1) as state,
            tc.tile_pool(name="work", bufs=2) as work,
            tc.tile_pool(name="small", bufs=6) as small,
            tc.tile_pool(name="pa", bufs=4, space="PSUM") as pa,
            tc.tile_pool(name="pb", bufs=2, space="PSUM") as pb,
            tc.tile_pool(name="pc", bufs=2, space="PSUM") as pc,
        ):
            ident = consts.tile([128, 128], f32)
            make_identity(nc, ident[:])
            ident_bf = consts.tile([128, 128], bf16)
            nc.vector.tensor_copy(out=ident_bf[:], in_=ident[:])
            eps_t = consts.tile([128, 1], f32)
            nc.vector.memset(eps_t[:], 1e-5)

            wq_sb = consts.tile([D_MODEL, n_layers, 4, 64], bf16)
            nc.sync.dma_start(out=wq_sb[:], in_=wq_d.rearrange("n k t m -> k n t m"))
            wk_sb = consts.tile([D_MODEL, n_layers, D_MODEL], bf16)
            nc.sync.dma_start(out=wk_sb[:], in_=wk_d.rearrange("n k m -> k n m"))
            wv_sb = consts.tile([D_MODEL, n_layers, D_MODEL], bf16)
            nc.sync.dma_start(out=wv_sb[:], in_=wv_d.rearrange("n k m -> k n m"))
            wo_sb = consts.tile([D_MODEL, n_layers, D_MODEL], f32)
            nc.sync.dma_start(out=wo_sb[:], in_=wo_d.rearrange("n k m -> k n m"))
            w1_sb = consts.tile([D_MODEL, n_layers, D_FF], bf16)
            nc.sync.dma_start(out=w1_sb[:], in_=w1_d.rearrange("n k m -> k n m"))
            w2_sb = consts.tile([128, n_layers, 4, D_MODEL], bf16)
            nc.sync.dma_start(out=w2_sb[:], in_=w2_d.rearrange("n c k m -> k n c m"))

            # Residual stream, resident in SBUF, one tile per group of 8
            # batches so groups don't serialize through a single tile's deps.
            GST = 8
            xs = []
            for gi in range(b_loc // GST):
                xg = state.tile([128, GST, D_MODEL], f32, tag=f"x{gi}")
                nc.sync.dma_start(
                    out=xg[:],
                    in_=x0_d[gi * GST : (gi + 1) * GST].rearrange("b l d -> l b d"),
                )
                xs.append(xg)

            def x_slice(b):
                return xs[b // GST][:, b % GST, :]
            # exp(biasT + mask): [128 j, b, h, 128 i]
            eb_sb = state.tile([128, b_loc, N_HEADS, L], bf16)
            for b in range(b_loc):
                nc.sync.dma_start(
                    out=eb_sb[:, b, :, :], in_=ebt_d[b].rearrange("j h i -> j h i")
                )

            G = 16 if b_loc % 16 == 0 else (8 if b_loc % 8 == 0 else 4)
            assert b_loc % G == 0
            for layer in range(n_layers):
                for g in range(b_loc // G):
                    bs = list(range(g * G, g * G + G))
                    # ---- A: x^T per batch -> xt_g [64, G, 128]
                    xt_g = work.tile([64, G, 128], bf16, tag="xt", bufs=1)
                    for j, b in enumerate(bs):
                        xt_ps = pa.tile([64, 128], f32, tag="pa")
                        nc.tensor.transpose(
                            out=xt_ps[:], in_=x_slice(b), identity=ident[:]
                        )
                        nc.scalar.copy(out=xt_g[:, j, :], in_=xt_ps[:])

                    # ---- B: K^T (batched), masked Q^T per head (batched), V
                    kt_g = work.tile([64, G, 128], bf16, tag="kt", bufs=1)
                    for cc in range(G // 4):
                        kt_ps = pa.tile([64, 512], f32, tag="pa")
                        nc.tensor.matmul(
                            out=kt_ps[:],
                            lhsT=wk_sb[:, layer, :],
                            rhs=xt_g[:, 4 * cc : 4 * cc + 4, :],
                            start=True, stop=True,
                        )
                        nc.vector.tensor_copy(
                            out=kt_g[:, 4 * cc : 4 * cc + 4, :], in_=kt_ps[:]
                        )
                    qt_g = work.tile([64, N_HEADS, G, 128], bf16, tag="qt", bufs=1)
                    for h in range(N_HEADS):
                        for cc in range(G // 4):
                            qt_ps = pa.tile([64, 512], f32, tag="pa")
                            nc.tensor.matmul(
                                out=qt_ps[:],
                                lhsT=wq_sb[:, layer, h, :],
                                rhs=xt_g[:, 4 * cc : 4 * cc + 4, :],
                                start=True, stop=True,
                            )
                            ceng = nc.vector.tensor_copy if h % 2 == 0 else nc.scalar.copy
                            ceng(
                                out=qt_g[:, h, 4 * cc : 4 * cc + 4, :], in_=qt_ps[:]
                            )
                    vp_g = work.tile([128, G, N_HEADS, 17], bf16, tag="vp")
                    nc.vector.memset(vp_g[:, :, :, 16:17], 1.0)
                    for j, b in enumerate(bs):
                        v_ps = pa.tile([128, 64], f32, tag="pa")
                        nc.tensor.matmul(
                            out=v_ps[:], lhsT=xt_g[:, j, :], rhs=wv_sb[:, layer, :],
                            start=True, stop=True,
                        )
                        nc.vector.tensor_copy(
                            out=vp_g[:, j, :, 0:16],
                            in_=v_ps[:].rearrange("p (h e) -> p h e", h=N_HEADS),
                        )

                    # ---- C: scores^T, exp (ACT), * exp-bias (GPSIMD)
                    at_g = work.tile([128, G, N_HEADS, 128], bf16, tag="at", bufs=1)
                    for j, b in enumerate(bs):
                        st_ps = pb.tile([128, N_HEADS, 128], f32, tag="pb")
                        nc.tensor.matmul(
                            out=st_ps[:],
                            lhsT=kt_g[:, j, :],
                            rhs=qt_g[:, :, j, :],
                            start=True, stop=True,
                        )
                        ex = work.tile([128, N_HEADS, 128], bf16, tag="ex", bufs=8)
                        nc.scalar.activation(
                            out=ex[:], in_=st_ps[:],
                            func=mybir.ActivationFunctionType.Exp,
                        )
                        eng = nc.gpsimd if j % 2 == 0 else nc.vector
                        eng.tensor_mul(
                            out=at_g[:, j], in0=ex[:], in1=eb_sb[:, b, :, :]
                        )

                    # ---- D: ctx' = A @ [V | 1]; normalize
                    ctx_g = work.tile([128, G, D_MODEL], f32, tag="ctxg")
                    for j, b in enumerate(bs):
                        ctx_ps = pc.tile([128, N_HEADS, 17], f32, tag="pc")
                        for h in range(N_HEADS):
                            nc.tensor.matmul(
                                out=ctx_ps[:, h, :],
                                lhsT=at_g[:, j, h, :], rhs=vp_g[:, j, h, :],
                                start=True, stop=True,
                            )
                        recip = small.tile([128, N_HEADS, 1], f32, tag="recip")
                        nc.vector.reciprocal(out=recip[:], in_=ctx_ps[:, :, 16:17])
                        nc.vector.tensor_mul(
                            out=ctx_g[:, j].rearrange("p (h e) -> p h e", h=N_HEADS),
                            in0=ctx_ps[:, :, 0:16],
                            in1=recip[:].to_broadcast([128, N_HEADS, 16]),
                        )

                    # ---- E: attn out proj + residual + LN -> x2_g
                    x2_g = work.tile([128, G, D_MODEL], f32, tag="x2g")
                    v1_g = work.tile([128, G, D_MODEL], f32, tag="v1g")
                    for j, b in enumerate(bs):
                        ctxt_ps = pa.tile([64, 128], f32, tag="pa")
                        nc.tensor.transpose(
                            out=ctxt_ps[:], in_=ctx_g[:, j, :], identity=ident[:]
                        )
                        ctxt = work.tile([64, 128], f32, tag="ctxt", bufs=8)
                        nc.vector.tensor_copy(out=ctxt[:], in_=ctxt_ps[:])
                        ao_ps = pa.tile([128, 64], f32, tag="pa")
                        nc.tensor.matmul(
                            out=ao_ps[:], lhsT=ctxt[:], rhs=wo_sb[:, layer, :],
                            start=True, stop=True,
                        )
                        nc.vector.tensor_add(
                            out=v1_g[:, j, :], in0=ao_ps[:], in1=x_slice(b)
                        )
                    _layernorm_group_emit(
                        nc, work, eps_t,
                        [x2_g[:, j, :] for j in range(G)], v1_g, G, mybir,
                    )

                    # ---- F: FFN (batched W1+relu, per-batch W2) + residual + LN
                    x2t_g = work.tile([64, G, 128], bf16, tag="x2t")
                    for j, b in enumerate(bs):
                        x2t_ps = pa.tile([64, 128], f32, tag="pa")
                        nc.tensor.transpose(
                            out=x2t_ps[:], in_=x2_g[:, j, :], identity=ident[:]
                        )
                        if j % 2 == 0:
                            nc.vector.tensor_copy(out=x2t_g[:, j, :], in_=x2t_ps[:])
                        else:
                            nc.scalar.copy(out=x2t_g[:, j, :], in_=x2t_ps[:])
                    ht_g = work.tile([128, 4, G, 128], bf16, tag="ht", bufs=1)
                    for c in range(4):
                        for cc in range(G // 4):
                            ht_ps = pc.tile([128, 512], f32, tag="pc")
                            nc.tensor.matmul(
                                out=ht_ps[:],
                                lhsT=w1_sb[:, layer, 128 * c : 128 * (c + 1)],
                                rhs=x2t_g[:, 4 * cc : 4 * cc + 4, :],
                                start=True, stop=True,
                            )
                            nc.scalar.activation(
                                out=ht_g[:, c, 4 * cc : 4 * cc + 4, :],
                                in_=ht_ps[:],
                                func=mybir.ActivationFunctionType.Relu,
                            )
                    v2_g = work.tile([128, G, D_MODEL], f32, tag="v2g")
                    for j, b in enumerate(bs):
                        y_ps = pa.tile([128, 64], f32, tag="pa")
                        for c in range(4):
                            nc.tensor.matmul(
                                out=y_ps[:],
                                lhsT=ht_g[:, c, j, :], rhs=w2_sb[:, layer, c, :],
                                start=(c == 0), stop=(c == 3),
                            )
                        nc.vector.tensor_add(
                            out=v2_g[:, j, :], in0=y_ps[:], in1=x2_g[:, j, :]
                        )
                    _layernorm_group_emit(
                        nc, work, eps_t,
                        [x_slice(b) for b in bs], v2_g, G, mybir,
                    )

            for b in range(b_loc):
                nc.sync.dma_start(out=out_d[b], in_=x_slice(b))

    _split_multi_waits(nc)
    return nc


def _pad_heads(w):
    """[n, 64, 64] -> [n, 64, 4, 64]: block h keeps only head h's 16 cols."""
    n = w.shape[0]
    out = np.zeros((n, D_MODEL, N_HEADS, D_MODEL), dtype=np.float32)
    for h in range(N_HEADS):
        sl = slice(D_K * h, D_K * (h + 1))
        out[:, :, h, sl] = w[:, :, sl]
    return out


def _host_prep(inputs):
    enc = np.asarray(inputs["enc_inputs"])
    deg = np.asarray(inputs["degree_s"])
    MD = np.asarray(inputs["MD"])
    src_emb = np.asarray(inputs["src_emb"], dtype=np.float32)
    deg_emb = np.asarray(inputs["deg_emb"], dtype=np.float32)
    md_emb = np.asarray(inputs["md_emb"], dtype=np.float32)

    x0 = src_emb[enc] + deg_emb[deg] + _positional_encoding()[None]
    x0 = x0.astype(np.float32)

    # bias[b,i,j,h] -> scores^T layout [b, j, h, i]; fold pad mask (key j
    # masked where enc[b, j] == 0) and exponentiate.
    bias_t = np.ascontiguousarray(md_emb[MD].transpose(0, 2, 3, 1))  # [B, j, h, i]
    mask = np.where(enc == 0, np.float32(-1e9), np.float32(0.0))  # over keys
    import ml_dtypes
    with np.errstate(under="ignore"):
        ebt = np.exp(bias_t + mask[:, :, None, None], dtype=np.float32)
    ebt = ebt.astype(ml_dtypes.bfloat16)

    import ml_dtypes as _mldw
    wq = _pad_heads(np.asarray(inputs["Wq"], dtype=np.float32) * SCALE).astype(
        _mldw.bfloat16
    )
    wk = np.asarray(inputs["Wk"], dtype=np.float32).astype(_mldw.bfloat16)
    wv = np.asarray(inputs["Wv"], dtype=np.float32).astype(_mldw.bfloat16)
    wo = np.asarray(inputs["Wo"], dtype=np.float32)
    w1 = np.asarray(inputs["W1"], dtype=np.float32).astype(_mldw.bfloat16)
    import ml_dtypes as _mld
    w2 = np.ascontiguousarray(
        np.asarray(inputs["W2"], dtype=np.float32).reshape(N_LAYERS, 4, 128, D_MODEL)
    ).astype(_mld.bfloat16)
    return x0, ebt, wq, wk, wv, wo, w1, w2


_NC_CACHE = {}


def run(inputs, trace=False, **spmd_kwargs):
    """Run on the 8 cores; returns (full_output, BassKernelResults)."""
    from concourse.bass_utils import run_bass_kernel_spmd

    x0, ebt, wq, wk, wv, wo, w1, w2 = _host_prep(inputs)

    if "nc" not in _NC_CACHE:
        _NC_CACHE["nc"] = build_nc()
    nc = _NC_CACHE["nc"]

    in_maps = []
    for c in range(N_CORES):
        sl = slice(c * B_LOC, (c + 1) * B_LOC)
        in_maps.append(
            dict(
                x0=np.ascontiguousarray(x0[sl]),
                ebt=np.ascontiguousarray(ebt[sl]),
                wq=wq, wk=wk, wv=wv, wo=wo, w1=w1, w2=w2,
            )
        )

    res = run_bass_kernel_spmd(
        nc, in_maps, core_ids=list(range(N_CORES)), trace=trace, **spmd_kwargs
    )
    out = np.concatenate([res.results[c]["out"] for c in range(N_CORES)], axis=0)
    return out.astype(np.float32), res


def kernel(**inputs):
    out, _ = run(inputs)
    return out


def _jit_single_core(nc):
    """Build a single-device jitted callable for nc (same program as SPMD)."""
    import jax
    from concourse import bass2jax
    from concourse import mybir

    bass2jax.install_neuronx_cc_hook()
    in_names, out_names, out_avals, zero_outs = [], [], [], []
    partition_name = nc.partition_id_tensor.name if nc.partition_id_tensor else None
    for alloc in nc.m.functions[0].allocations:
        if not isinstance(alloc, mybir.MemoryLocationSet):
            continue
        name = alloc.memorylocations[0].name
        if alloc.kind == "ExternalInput":
            if name != partition_name:
                in_names.append(name)
        elif alloc.kind == "ExternalOutput":
            out_names.append(name)
            shape = tuple(alloc.tensor_shape)
            dtype = mybir.dt.np(alloc.dtype)
            out_avals.append(jax.core.ShapedArray(shape, dtype))
            zero_outs.append(np.zeros(shape, dtype))
    n_params = len(in_names)
    all_names = in_names + out_names + ([partition_name] if partition_name else [])
    donate = tuple(range(n_params, n_params + len(out_names)))

    def _body(*args):
        operands = list(args)
        if partition_name is not None:
            operands.append(bass2jax.partition_id_tensor())
        outs = bass2jax._bass_exec_p.bind(
            *operands,
            out_avals=tuple(out_avals),
            in_names=tuple(all_names),
            out_names=tuple(out_names),
            lowering_input_output_aliases=(),
            sim_require_finite=True,
            sim_require_nnan=True,
            nc=nc,
        )
        return tuple(outs)

    jfn = jax.jit(_body, donate_argnums=donate, keep_unused=True)
    return jfn, in_names, zero_outs


def bench_marginal(inputs, iters=24, reps=2):
    """Per-execution device time via async dispatch pipelining: issue
    `iters` executions without blocking (independent submissions pipeline on
    the core), block once at the end; marginal over 1-call runs cancels the
    ~90 ms axon dispatch overhead."""
    import time

    import jax

    x0, ebt, wq, wk, wv, wo, w1, w2 = _host_prep(inputs)
    if "nc" not in _NC_CACHE:
        _NC_CACHE["nc"] = build_nc()
    nc = _NC_CACHE["nc"]
    in_map = dict(
        x0=np.ascontiguousarray(x0[:B_LOC]),
        ebt=np.ascontiguousarray(ebt[:B_LOC]),
        wq=wq, wk=wk, wv=wv, wo=wo, w1=w1, w2=w2,
    )
    jfn, in_names, zero_outs = _jit_single_core(nc)
    dev = jax.devices()[0]
    ins_dev = [jax.device_put(np.asarray(in_map[n]), dev) for n in in_names]
    n_zsets = (iters + 2) * reps + 4
    zsets = [
        [jax.device_put(z.copy(), dev) for z in zero_outs] for _ in range(n_zsets)
    ]
    jax.block_until_ready(zsets)
    jax.block_until_ready(ins_dev)
    state = {"zi": 0}

    def run_m(m):
        outs = []
        t0 = time.perf_counter()
        for _ in range(m):
            outs.append(jfn(*ins_dev, *zsets[state["zi"]]))
            state["zi"] += 1
        jax.block_until_ready(outs)
        return time.perf_counter() - t0

    run_m(1)  # warm (compiles)
    t1s, tns = [], []
    for _ in range(reps):
        t1s.append(run_m(1))
        tns.append(run_m(iters))
    marginal_ns = (min(tns) - min(t1s)) / (iters - 1) * 1e9
    return dict(
        est_exec_ns=marginal_ns,
        t1_ns=min(t1s) * 1e9,
        tn_ns=min(tns) * 1e9,
        t1s=t1s,
        tns=tns,
        iters=iters,
    )


def bench(inputs, iters=8):
    """Estimate on-device exec time: repeated single-core runs with
    device-resident inputs, minus a tiny-kernel dispatch baseline."""
    import time

    import jax

    x0, ebt, wq, wk, wv, wo, w1, w2 = _host_prep(inputs)
    if "nc" not in _NC_CACHE:
        _NC_CACHE["nc"] = build_nc()
    nc = _NC_CACHE["nc"]
    in_map = dict(
        x0=np.ascontiguousarray(x0[:B_LOC]),
        ebt=np.ascontiguousarray(ebt[:B_LOC]),
        wq=wq, wk=wk, wv=wv, wo=wo, w1=w1, w2=w2,
    )

    dev = jax.devices()[0]

    def time_kernel(nc_, im):
        jfn, in_names, zero_outs = _jit_single_core(nc_)
        ins_dev = [jax.device_put(np.asarray(im[n]), dev) for n in in_names]
        zero_sets = [
            [jax.device_put(z.copy(), dev) for z in zero_outs] for _ in range(iters + 1)
        ]
        jax.block_until_ready(ins_dev)
        jax.block_until_ready(zero_sets)
        # warmup (compiles)
        out = jfn(*ins_dev, *zero_sets[0])
        jax.block_until_ready(out)
        ts = []
        for i in range(iters):
            t0 = time.perf_counter()
            out = jfn(*ins_dev, *zero_sets[i + 1])
            jax.block_until_ready(out)
            ts.append(time.perf_counter() - t0)
        return min(ts), ts

    t_full, ts_full = time_kernel(nc, in_map)

    # dispatch-overhead baseline: trivial 1-tile kernel
    if "nc_tiny" not in _NC_CACHE:
        import concourse.bass as bass
        import concourse.mybir as mybir
        import concourse.tile as tile

        nct = bass.Bass("TRN2", target_bir_lowering=False, debug=False)
        a_d = nct.dram_tensor("a", [128, 128], mybir.dt.float32, kind="ExternalInput")
        b_d = nct.dram_tensor("b", [128, 128], mybir.dt.float32, kind="ExternalOutput")
        with tile.TileContext(nct) as tc:
            with tc.tile_pool(name="p", bufs=1) as p:
                t = p.tile([128, 128], mybir.dt.float32)
                nct.sync.dma_start(out=t[:], in_=a_d.ap())
                nct.scalar.mul(out=t[:], in_=t[:], mul=2.0)
                nct.sync.dma_start(out=b_d.ap(), in_=t[:])
        _split_multi_waits(nct)
        _NC_CACHE["nc_tiny"] = nct
    t_tiny, ts_tiny = time_kernel(
        _NC_CACHE["nc_tiny"], {"a": np.ones((128, 128), np.float32)}
    )

    est_ns = max(0.0, (t_full - t_tiny)) * 1e9
    return dict(
        est_exec_ns=est_ns,
        t_full_ns=t_full * 1e9,
        t_tiny_ns=t_tiny * 1e9,
        ts_full=ts_full,
        ts_tiny=ts_tiny,
    )


if __name__ == "__main__":
    rng = np.random.default_rng(0)
    print("kernel module ok")

